# revision 45
# baseline (speedup 1.0000x reference)
"""3-layer GAT (PyG GATConv semantics) on 8 Trainium2 NeuronCores — v4.

Strategy (dst-sharded, CSR-ELL, batched dma_gather, chunked collectives):
- Nodes sorted by in-degree, grouped into 20 degree-bands of 1024; band g gives
  one 128-node block to each of the 8 cores with a shared column count
  Tg[g] = max degree in the band. Edge layout per block is ELL: partition =
  dst slot, free column j = j-th incoming edge (~8% padding).
- Table rows are numbered CHUNK-MAJOR (4 chunks of 5 bands) so each chunk's
  AllGather reads/writes contiguous rows; chunk AGs of layer l+1 fire while
  layer l's edge phase is still running (dense of l+1 is interleaved per
  chunk into l's edge phase).
- Dense: h_aug = h @ [W | ws | 0.2ws | wd | 0.2wd] per block ([128, 272] PSUM);
  cols 0:264 ([h | asrc | 0.2asrc]) go fp16 into the 768B-stride table row,
  cols 264:272 (adst, 0.2adst) stay SBUF-resident.
- Edge phase per block: batched InstDMAGatherAnt (mlp GPSIMD library, int16
  indices, <=1024 idxs/instruction, round-robin over 4 SWDGE queues) pulls rows
  into ELL position. leaky = max(asrc+adst, 0.2asrc+0.2adst) (prescaled, no
  tensor_scalar), + pad mask, exp on the scalar engine with a -ln(64) bias
  (fp16 fold-overflow guard; cancels in the softmax ratio). Messages multiply
  in place; aggregation + denominator = free-axis halving-tree fold. Stages are
  software-pipelined across blocks (A(g) issued before B(g-1)) and the
  normalize+bias+ELU tail is batched per 5-block chunk.
- Layer 3 (heads=1, C=1): same scheme over a 256B-row scalar table; h3[dst]
  comes from the resident dense output.

The walrus in this toolchain accepts only ONE sync wait per instruction;
BassOneWait splits Tile-generated multi-waits into single-wait EventSemaphore
ops at serialization.
"""
import numpy as np
from contextlib import ExitStack

import orjson
import concourse.bass as bass
import concourse.tile as tile
from concourse import mybir, library_config
from concourse.library_overlay import lower_extended_insts
from concourse.bass_utils import run_bass_kernel_spmd

# problem constants (fixed by the harness's setup_inputs)
N_NODES = 20000
N_EDGES = 320000
IN_DIM = 128
HID = 64
HEADS = 4
HC = HEADS * HID          # 256
AUG = HC + 4 * HEADS      # 272 = h | ws | 0.2ws | wd | 0.2wd
TABW = HC + 2 * HEADS     # 264 = table row payload: h | asrc | 0.2asrc
ROWE = 384                # table row stride in fp16 elems (768B)
ROW3 = 128                # layer-3 table row stride in fp16 elems (256B)
NEG = 0.2
NCORES = 8
P = 128
NBLK = 20                 # dst blocks per core (degree bands)
BPC = 5                   # bands per AG chunk
NCHUNK = NBLK // BPC      # 4
SLOTS = NBLK * P          # 2560 slots per core
TOT_SLOTS = SLOTS * NCORES
NPAD = TOT_SLOTS          # 20480 (480 pad slots)
CROWS = NCORES * BPC * P  # 5120 table rows per chunk
CSTRIDE = CROWS           # no sacrificial row: Local AG, offset-0 outputs
TOT_TAB = NCHUNK * CSTRIDE
GMAX = 1024               # max indices per dma_gather (SWDGE ring limit)
NQUEUES = 4               # SWDGE queues (round-robin gathers across Q7 rings)
UMAX = 24                 # max ELL columns per gather unit (bounds hg tile)
MASKV = -30000.0          # additive mask for ELL pad columns
EXP_SHIFT = -4.158883083359672   # ln(1/64): guards fp16 fold overflow

F32 = mybir.dt.float32
F16 = mybir.dt.float16
I16 = mybir.dt.int16

AF = mybir.ActivationFunctionType
OP = mybir.AluOpType


def _split_multiwaits(bir: bytes) -> bytes:
    """Walrus here allows only 1 sync wait per instruction -> hoist extras onto
    same-engine EventSemaphore waits (dedup repeated ge-waits per engine; sems
    are monotonic within the block, so a repeated >= wait is a no-op)."""
    j = orjson.loads(bir)
    ctr = 0
    for fn in j["functions"]:
        for blk in fn["blocks"]:
            out_l = []
            last_wait = {}
            for ins in blk["instructions"]:
                eng = ins.get("engine")
                si = ins.get("sync_info")
                ow = (si or {}).get("on_wait") or []
                keep = 1
                if len(ow) > keep:
                    seen = last_wait.setdefault(eng, set())
                    for w in ow[:len(ow) - keep]:
                        key = (w.get("id"), w.get("wait_mode"), w.get("wait_value"))
                        if w.get("wait_mode") == "sem-ge-imm":
                            if key in seen:
                                continue
                            seen.add(key)
                        ctr += 1
                        out_l.append({
                            "engine": eng, "ins": [], "outs": [],
                            "name": f"mwsplit-{ctr}", "opcode": "EventSemaphore",
                            "sync_info": {"on_update": [], "on_wait": [w]},
                        })
                    si["on_wait"] = ow[len(ow) - keep:]
                out_l.append(ins)
            blk["instructions"] = out_l
    return orjson.dumps(j)


class BassOneWait(bass.Bass):
    def to_json_bytes(self):
        return _split_multiwaits(super().to_json_bytes())


# ---------------------------------------------------------------- host prep

def _row_of(c, g, p):
    """Chunk-major global table row of (core c, band g, slot p)."""
    k = g // BPC
    return k * CSTRIDE + c * (BPC * P) + (g % BPC) * P + p


def _preprocess(edge_index):
    """Degree-sorted band assignment + ELL edge layout + gather index arrays."""
    src = np.asarray(edge_index[0], dtype=np.int64)
    dst = np.asarray(edge_index[1], dtype=np.int64)
    loops = np.arange(N_NODES, dtype=np.int64)
    src = np.concatenate([src, loops])
    dst = np.concatenate([dst, loops])

    deg = np.zeros(NPAD, np.int64)
    deg[:N_NODES] = np.bincount(dst, minlength=N_NODES)

    order = np.argsort(-deg, kind="stable")          # rank -> node
    rank = np.empty(NPAD, np.int64)
    rank[order] = np.arange(NPAD)

    Tg = tuple(max(int(deg[order[g * 1024]]), 1) for g in range(NBLK))
    goff = np.concatenate([[0], np.cumsum(Tg)])
    NTT = int(goff[-1])

    g_of = rank // 1024
    w = rank % 1024
    c_of = w // P
    p_of = w % P
    grow = _row_of(c_of, g_of, p_of)                 # node -> global table row

    eord = np.argsort(dst, kind="stable")
    dsts = dst[eord]
    srcs = src[eord]
    starts = np.searchsorted(dsts, np.arange(N_NODES + 1))
    j = np.arange(len(dsts)) - starts[dsts]

    ec = c_of[dsts]
    ep = p_of[dsts]
    ecol = goff[g_of[dsts]] + j

    idx_flat = np.zeros((NCORES, NTT * P), np.int16)
    idx_flat[ec, ecol * P + ep] = grow[srcs].astype(np.int16)
    mask = np.full((NCORES, P, NTT), MASKV, np.float16)
    mask[ec, ep, ecol] = 0.0

    idx_w = np.empty((NCORES, P, NTT * 8), np.int16)
    for c in range(NCORES):
        w16 = idx_flat[c].reshape(NTT * 8, 16).T     # [16, NTT*8]
        idx_w[c] = np.tile(w16, (8, 1))

    return Tg, NTT, order, idx_w, mask


def _aug_weights(W, a_src, a_dst, heads, hid):
    """[W | ws | 0.2ws | wd | 0.2wd]; ws[:,h] = W[:, h*hid:(h+1)*hid] @ a_src[h]."""
    cin = W.shape[0]
    ws = np.zeros((cin, heads), np.float32)
    wd = np.zeros((cin, heads), np.float32)
    for h in range(heads):
        blk = W[:, h * hid:(h + 1) * hid]
        ws[:, h] = blk @ a_src[h]
        wd[:, h] = blk @ a_dst[h]
    return np.concatenate([W, ws, NEG * ws, wd, NEG * wd], axis=1).astype(np.float32)


# ---------------------------------------------------------------- device kernel

def _build(Tg):
    NTT = sum(Tg)
    TMAX = max(Tg)
    goff = [0]
    for t in Tg:
        goff.append(goff[-1] + t)

    nc = BassOneWait(num_swdge_queues=NQUEUES)
    dp = nc.declare_dram_parameter
    x_in = dp("x_in", [SLOTS, IN_DIM], F32, isOutput=False)
    idx_in = dp("idx_in", [P, NTT * 8], I16, isOutput=False)
    mask_in = dp("mask_in", [P, NTT], F16, isOutput=False)
    wa1_in = dp("wa1_in", [IN_DIM, AUG], F32, isOutput=False)
    wa2_in = dp("wa2_in", [HC, AUG], F32, isOutput=False)
    w3_in = dp("w3_in", [1, HC], F32, isOutput=False)
    c3_in = dp("c3_in", [1, 4], F32, isOutput=False)   # a_src3, a_dst3, b3, 0.2*a_src3
    b1_in = dp("b1_in", [1, HC], F32, isOutput=False)
    b2_in = dp("b2_in", [1, HC], F32, isOutput=False)
    ident_in = dp("ident_in", [P, P], F32, isOutput=False)
    out_p = dp("out_p", [P, NBLK], F32, isOutput=True)

    tab_sh = [nc.dram_tensor(f"tab_sh{l}", [SLOTS, ROWE], F16) for l in (1, 2)]
    tab_full = [nc.dram_tensor(f"tab_full{l}", [TOT_TAB, ROWE], F16)
                for l in (1, 2)]
    tab3_sh = nc.dram_tensor("tab3_sh", [SLOTS, ROW3], F16)
    tab3_full = nc.dram_tensor("tab3_full", [TOT_TAB, ROW3], F16)

    groups = [list(range(NCORES))]

    with tile.TileContext(nc) as tc, ExitStack() as ctx:
        consts = ctx.enter_context(tc.tile_pool(name="consts", bufs=1))
        meta = ctx.enter_context(tc.tile_pool(name="meta", bufs=1))
        state = ctx.enter_context(tc.tile_pool(name="state", bufs=1))
        gpool = ctx.enter_context(tc.tile_pool(name="gpool", bufs=4))
        sm = ctx.enter_context(tc.tile_pool(name="sm", bufs=4))
        tl = ctx.enter_context(tc.tile_pool(name="tl", bufs=2))
        psd = ctx.enter_context(tc.tile_pool(name="psd", bufs=2, space="PSUM"))
        pst = ctx.enter_context(tc.tile_pool(name="pst", bufs=2, space="PSUM"))

        nc.gpsimd.load_library(library_config.mlp)

        # ---- constants / metadata
        ident = consts.tile([P, P], F32)
        nc.sync.dma_start(out=ident, in_=ident_in[:])
        wa1 = consts.tile([P, AUG], F32)
        nc.sync.dma_start(out=wa1, in_=wa1_in[:])
        wa2 = consts.tile([P, 2, AUG], F32)
        nc.sync.dma_start(out=wa2, in_=wa2_in.rearrange("(j p) a -> p j a", p=P))

        def rep_load(name, srct, n, dt):
            t = consts.tile([P, n], dt, tag=name)
            bc = bass.AP(tensor=srct.tensor, offset=0, ap=[[0, P], [1, n]])
            nc.sync.dma_start(out=t, in_=bc)
            return t
        w3r = rep_load("w3r", w3_in[:], HC, F32)
        c3 = rep_load("c3", c3_in[:], 4, F32)
        b1r = rep_load("b1r", b1_in[:], HC, F32)
        b2r = rep_load("b2r", b2_in[:], HC, F32)

        eshift = consts.tile([P, 1], F32, tag="eshift")
        nc.vector.memset(eshift[:], EXP_SHIFT)

        idx = meta.tile([P, NTT * 8], I16)
        nc.sync.dma_start(out=idx, in_=idx_in[:])
        msk = meta.tile([P, NTT], F16)
        nc.sync.dma_start(out=msk, in_=mask_in[:])

        xin = state.tile([P, NBLK, IN_DIM], F32)
        nc.sync.dma_start(out=xin, in_=x_in.rearrange("(b p) d -> p b d", p=P))

        hprev = state.tile([P, NBLK, HC], F32)
        hprev2 = state.tile([P, NBLK, HC], F32)
        hT = state.tile([P, 2 * NBLK, P], F32)
        adl0 = state.tile([P, NBLK, 2 * HEADS], F32, tag="adl0")
        adl1 = state.tile([P, NBLK, 2 * HEADS], F32, tag="adl1")
        adls = [adl0, adl1]
        coll = state.tile([P, NBLK, TABW], F16)      # fold results (num|den)
        coll3 = state.tile([P, NBLK, 2], F16)
        h3sb = state.tile([P, NBLK, 1], F32)
        h316 = state.tile([P, NBLK, 1], F16)
        hd3 = state.tile([P, NBLK, 1], F32)          # a_dst3 * h3
        hd3l = state.tile([P, NBLK, 1], F32)         # 0.2 * a_dst3 * h3
        outsb = state.tile([P, NBLK], F32)

        def ap_of(t_slice, ap):
            return bass.AP(tensor=t_slice.tensor, offset=t_slice.offset, ap=ap)

        def transpose_into(src_view, dst_col):
            tp = pst.tile([P, P], F32, tag="tr")
            nc.tensor.transpose(out=tp, in_=src_view, identity=ident)
            nc.vector.tensor_copy(out=hT[:, dst_col, :], in_=tp)

        def dense_block(lidx, g):
            """h_aug for block g of layer lidx (0/1) -> table row + resident adl."""
            if lidx == 0:
                transpose_into(xin[:, g, :], g)
                cin_tiles = 1
            else:
                transpose_into(hprev[:, g, 0:P], 2 * g)
                transpose_into(hprev[:, g, P:HC], 2 * g + 1)
                cin_tiles = 2
            ps = psd.tile([P, AUG], F32, tag="dense")
            for jj in range(cin_tiles):
                lhsT = hT[:, cin_tiles * g + jj, :]
                rhs = wa1[:, :] if lidx == 0 else wa2[:, jj, :]
                nc.tensor.matmul(ps, lhsT, rhs,
                                 start=(jj == 0), stop=(jj == cin_tiles - 1))
            tabt = sm.tile([P, TABW], F16, tag="tabt")
            nc.vector.tensor_copy(out=tabt, in_=ps[:, 0:TABW])
            nc.sync.dma_start(
                out=tab_sh[lidx].rearrange("(g p) e -> p g e", p=P)[:, g, 0:TABW],
                in_=tabt)
            nc.vector.tensor_copy(out=adls[lidx][:, g, :], in_=ps[:, TABW:AUG])

        def ag_chunk(tin, tout, k):
            cc = nc.gpsimd.collective_compute(
                "AllGather", OP.bypass, replica_groups=groups,
                ins=[tin[BPC * P * k: BPC * P * (k + 1), :]],
                outs=[tout[CSTRIDE * k: CSTRIDE * k + CROWS, :]])
            return cc

        nidx_regs = {}
        qctr = [0]

        def gathers(table, elem, out_view, g, u0, u1):
            c0 = u0
            while c0 < u1:
                ncols = min(GMAX // P, u1 - c0)
                if ncols not in nidx_regs:
                    nidx_regs[ncols] = nc.gpsimd.to_reg(P * ncols)
                col = goff[g] + c0
                nc.gpsimd.dma_gather(
                    out_ap=out_view[:, c0 - u0:c0 - u0 + ncols, :],
                    in_ap=table[:],
                    idxs_ap=idx[:, 8 * col: 8 * (col + ncols)],
                    num_idxs=P * ncols, num_idxs_reg=nidx_regs[ncols],
                    elem_size=elem, queue_num=qctr[0])
                qctr[0] = (qctr[0] + 1) % NQUEUES
                c0 += ncols

        # split bands into gather units of <= UMAX columns
        units = []                         # (g, u0, u1, last_of_band)
        for g in range(NBLK):
            T = Tg[g]
            nu = -(-T // UMAX)
            step = -(-T // nu)
            c0 = 0
            while c0 < T:
                c1 = min(c0 + step, T)
                units.append((g, c0, c1, c1 == T))
                c0 = c1

        def fold_cols(t, T, w):
            n = T
            while n > 1:
                if n % 2 == 1:
                    nc.vector.tensor_tensor(
                        out=t[:, 0:1, 0:w], in0=t[:, 0:1, 0:w],
                        in1=t[:, n - 1:n, 0:w], op=OP.add)
                    n -= 1
                h = n // 2
                nc.vector.tensor_tensor(
                    out=t[:, 0:h, 0:w], in0=t[:, 0:h, 0:w],
                    in1=t[:, h:2 * h, 0:w], op=OP.add)
                n = h

        # ---------------- layer 1/2 edge phase stages (per unit)
        def stage_a(lidx, g, u0, u1):
            T = u1 - u0
            adl = adls[lidx]
            hg = gpool.tile([P, UMAX, ROWE], F16, tag="hg")
            gathers(tab_full[lidx], ROWE, hg, g, u0, u1)
            e = sm.tile([P, UMAX, HEADS], F32, tag="e")
            e2 = sm.tile([P, UMAX, HEADS], F32, tag="e2")
            adl_b = ap_of(adl[:, g, 0:HEADS], [list(adl.ap[0]), [0, T], [1, HEADS]])
            adl2_b = ap_of(adl[:, g, HEADS:2 * HEADS],
                           [list(adl.ap[0]), [0, T], [1, HEADS]])
            nc.vector.tensor_tensor(out=e[:, :T, :], in0=hg[:, :T, HC:HC + HEADS],
                                    in1=adl_b, op=OP.add)
            nc.vector.tensor_tensor(out=e2[:, :T, :],
                                    in0=hg[:, :T, HC + HEADS:HC + 2 * HEADS],
                                    in1=adl2_b, op=OP.add)
            nc.vector.tensor_tensor(out=e[:, :T, :], in0=e[:, :T, :],
                                    in1=e2[:, :T, :], op=OP.max)
            msk_b = ap_of(msk[:, goff[g] + u0:goff[g] + u1],
                          [list(msk.ap[0]), [msk.ap[1][0], T], [0, HEADS]])
            nc.vector.tensor_tensor(out=e[:, :T, :], in0=e[:, :T, :],
                                    in1=msk_b, op=OP.add)
            exf = sm.tile([P, UMAX, HEADS], F16, tag="exf")
            nc.scalar.activation(out=exf[:, :T, :], in_=e[:, :T, :],
                                 func=AF.Exp, bias=eshift[:, :])
            return hg, exf

        def stage_b(g, u0, u1, hg, exf):
            T = u1 - u0
            exf_b = ap_of(exf[:, 0:T, :],
                          [list(exf.ap[0]), [HEADS, T], [1, HEADS], [0, HID]])
            hg4 = hg[:, 0:T, 0:HC].rearrange("p t (h c) -> p t h c", h=HEADS)
            nc.vector.tensor_tensor(out=hg4, in0=hg4, in1=exf_b, op=OP.mult)
            nc.vector.tensor_copy(out=hg[:, :T, HC:HC + HEADS], in_=exf[:, :T, :])
            fold_cols(hg, T, TABW)
            if u0 == 0:
                nc.vector.tensor_copy(out=coll[:, g, :], in_=hg[:, 0, 0:TABW])
            else:
                nc.vector.tensor_tensor(out=coll[:, g, :], in0=coll[:, g, :],
                                        in1=hg[:, 0, 0:TABW], op=OP.add)

        def tail_chunk(k, brow, hout):
            s = slice(BPC * k, BPC * (k + 1))
            den = tl.tile([P, BPC, HEADS], F32, tag="den")
            nc.vector.tensor_scalar_max(den, coll[:, s, HC:HC + HEADS], 1e-30)
            rec = tl.tile([P, BPC, HEADS], F32, tag="rec")
            nc.vector.reciprocal(out=rec, in_=den)
            rec_b = ap_of(rec[:, :, :],
                          [list(rec.ap[0]), [HEADS, BPC], [1, HEADS], [0, HID]])
            hn = tl.tile([P, BPC, HC], F32, tag="hn")
            nc.vector.tensor_tensor(
                out=hn.rearrange("p b (h c) -> p b h c", h=HEADS),
                in0=coll[:, s, 0:HC].rearrange("p b (h c) -> p b h c", h=HEADS),
                in1=rec_b, op=OP.mult)
            brow_b = ap_of(brow[:, :], [list(brow.ap[0]), [0, BPC], [1, HC]])
            nc.vector.tensor_tensor(out=hn, in0=hn, in1=brow_b, op=OP.add)
            r = tl.tile([P, BPC, HC], F32, tag="r")
            nc.vector.tensor_scalar_min(r, hn, 0.0)
            nc.scalar.activation(out=r, in_=r, func=AF.Exp)
            nc.vector.tensor_scalar_max(hn, hn, 0.0)
            nc.vector.tensor_tensor(out=hn, in0=hn, in1=r, op=OP.add)
            nc.vector.tensor_scalar_add(hout[:, s, :], hn, -1.0)

        def edge_layer(lidx, brow, hout, post_chunk):
            pend = []
            def drain_one():
                up, hgp, exfp = pend.pop(0)
                stage_b(up[0], up[1], up[2], hgp, exfp)
                if up[3] and up[0] % BPC == BPC - 1:
                    k = up[0] // BPC
                    tail_chunk(k, brow, hout)
                    post_chunk(k)
            for u in units:
                cur = stage_a(lidx, u[0], u[1], u[2])
                pend.append((u, cur[0], cur[1]))
                if len(pend) > 3:
                    drain_one()
            while pend:
                drain_one()

        # ================= layer 1 dense + chunked AG
        for k in range(NCHUNK):
            for g in range(BPC * k, BPC * (k + 1)):
                dense_block(0, g)
            ag_chunk(tab_sh[0], tab_full[0], k)

        # ================= layer 1 edge (+ layer 2 dense/AG interleaved)
        def post1(k):
            for g in range(BPC * k, BPC * (k + 1)):
                dense_block(1, g)
            ag_chunk(tab_sh[1], tab_full[1], k)
        edge_layer(0, b1r, hprev, post1)

        # ================= layer 2 edge (+ layer 3 dense/AG interleaved)
        def post2(k):
            s = slice(BPC * k, BPC * (k + 1))
            for g in range(BPC * k, BPC * (k + 1)):
                tmp = sm.tile([P, HC], F32, tag="l3tmp")
                nc.vector.tensor_tensor(out=tmp, in0=hprev2[:, g, :], in1=w3r,
                                        op=OP.mult)
                nc.vector.tensor_reduce(out=h3sb[:, g, :], in_=tmp,
                                        axis=mybir.AxisListType.X, op=OP.add)
            nc.vector.tensor_copy(out=h316[:, s, :], in_=h3sb[:, s, :])
            ad3_b = ap_of(c3[:, 1:2], [list(c3.ap[0]), [0, BPC], [0, 1]])
            nc.vector.tensor_tensor(out=hd3[:, s, :], in0=h3sb[:, s, :],
                                    in1=ad3_b, op=OP.mult)
            nc.vector.tensor_scalar_mul(hd3l[:, s, :], hd3[:, s, :], NEG)
            nc.sync.dma_start(
                out=tab3_sh.rearrange("(g p) e -> p g e", p=P)[:, s, 0:1],
                in_=h316[:, s, :])
            ag_chunk(tab3_sh, tab3_full, k)
        edge_layer(1, b2r, hprev2, post2)

        # ================= layer 3 edge phase (pipelined A/B, batched tail)
        def stage_a3(g, u0, u1):
            T = u1 - u0
            hg = gpool.tile([P, UMAX, ROWE], F16, tag="hg")
            g3 = hg.rearrange("p t (x e) -> p (t x) e", x=ROWE // ROW3)
            gathers(tab3_full, ROW3, g3, g, u0, u1)
            e3 = sm.tile([P, UMAX, 1], F32, tag="e3")
            e3l = sm.tile([P, UMAX, 1], F32, tag="e3l")
            as3_b = ap_of(c3[:, 0:1], [list(c3.ap[0]), [0, T], [0, 1]])
            as3l_b = ap_of(c3[:, 3:4], [list(c3.ap[0]), [0, T], [0, 1]])
            nc.vector.tensor_tensor(out=e3[:, :T, :], in0=g3[:, :T, 0:1],
                                    in1=as3_b, op=OP.mult)
            nc.vector.tensor_tensor(out=e3l[:, :T, :], in0=g3[:, :T, 0:1],
                                    in1=as3l_b, op=OP.mult)
            hd3_b = ap_of(hd3[:, g, :], [list(hd3.ap[0]), [0, T], [1, 1]])
            hd3l_b = ap_of(hd3l[:, g, :], [list(hd3l.ap[0]), [0, T], [1, 1]])
            nc.vector.tensor_tensor(out=e3[:, :T, :], in0=e3[:, :T, :],
                                    in1=hd3_b, op=OP.add)
            nc.vector.tensor_tensor(out=e3l[:, :T, :], in0=e3l[:, :T, :],
                                    in1=hd3l_b, op=OP.add)
            nc.vector.tensor_tensor(out=e3[:, :T, :], in0=e3[:, :T, :],
                                    in1=e3l[:, :T, :], op=OP.max)
            msk_b = ap_of(msk[:, goff[g] + u0:goff[g] + u1],
                          [list(msk.ap[0]), [msk.ap[1][0], T], [0, 1]])
            nc.vector.tensor_tensor(out=e3[:, :T, :], in0=e3[:, :T, :],
                                    in1=msk_b, op=OP.add)
            ex3 = sm.tile([P, UMAX, 1], F16, tag="ex3")
            nc.scalar.activation(out=ex3[:, :T, :], in_=e3[:, :T, :],
                                 func=AF.Exp, bias=eshift[:, :])
            return hg, g3, ex3

        def stage_b3(g, u0, u1, g3, ex3):
            T = u1 - u0
            nc.vector.tensor_tensor(out=g3[:, :T, 0:1], in0=g3[:, :T, 0:1],
                                    in1=ex3[:, :T, :], op=OP.mult)
            nc.vector.tensor_copy(out=g3[:, :T, 1:2], in_=ex3[:, :T, :])
            fold_cols(g3, T, 2)
            if u0 == 0:
                nc.vector.tensor_copy(out=coll3[:, g, :], in_=g3[:, 0, 0:2])
            else:
                nc.vector.tensor_tensor(out=coll3[:, g, :], in0=coll3[:, g, :],
                                        in1=g3[:, 0, 0:2], op=OP.add)

        pend3 = []
        for u in units:
            cur = stage_a3(u[0], u[1], u[2])
            pend3.append((u, cur[1], cur[2]))
            if len(pend3) > 3:
                up, g3p, ex3p = pend3.pop(0)
                stage_b3(up[0], up[1], up[2], g3p, ex3p)
        while pend3:
            up, g3p, ex3p = pend3.pop(0)
            stage_b3(up[0], up[1], up[2], g3p, ex3p)

        den3 = tl.tile([P, NBLK, 1], F32, tag="den3")
        nc.vector.tensor_scalar_max(den3, coll3[:, :, 1:2], 1e-30)
        rec3 = tl.tile([P, NBLK, 1], F32, tag="rec3")
        nc.vector.reciprocal(out=rec3, in_=den3)
        outsb3 = ap_of(outsb[:, :], [list(outsb.ap[0]), [1, NBLK], [1, 1]])
        nc.vector.tensor_tensor(out=outsb3, in0=coll3[:, :, 0:1], in1=rec3,
                                op=OP.mult)
        b3_b = ap_of(c3[:, 2:3], [list(c3.ap[0]), [0, NBLK]])
        nc.vector.tensor_tensor(out=outsb, in0=outsb, in1=b3_b, op=OP.add)
        nc.sync.dma_start(out=out_p[:], in_=outsb)

    lower_extended_insts(nc)
    return nc


_CACHE = {}


def kernel(x, edge_index, W1, a_src1, a_dst1, b1, W2, a_src2, a_dst2, b2,
           W3, a_src3, a_dst3, b3):
    Tg, NTT, order, idx_w, mask = _preprocess(np.asarray(edge_index))

    wa1 = _aug_weights(np.asarray(W1, np.float32), np.asarray(a_src1, np.float32),
                       np.asarray(a_dst1, np.float32), HEADS, HID)
    wa2 = _aug_weights(np.asarray(W2, np.float32), np.asarray(a_src2, np.float32),
                       np.asarray(a_dst2, np.float32), HEADS, HID)
    w3 = np.asarray(W3, np.float32).reshape(1, HC)
    a_s3 = float(np.asarray(a_src3).reshape(-1)[0])
    c3 = np.array([[a_s3,
                    float(np.asarray(a_dst3).reshape(-1)[0]),
                    float(np.asarray(b3).reshape(-1)[0]), NEG * a_s3]], np.float32)
    b1r = np.asarray(b1, np.float32).reshape(1, HC)
    b2r = np.asarray(b2, np.float32).reshape(1, HC)

    x = np.asarray(x, np.float32)
    in_maps = []
    for c in range(NCORES):
        r = (np.arange(NBLK)[:, None] * 1024 + c * P + np.arange(P)[None, :])
        nodes = order[r.reshape(-1)]                 # [2560] slot-major
        xs = np.zeros((SLOTS, IN_DIM), np.float32)
        valid = nodes < N_NODES
        xs[valid] = x[nodes[valid]]
        in_maps.append({
            "x_in": xs,
            "idx_in": idx_w[c], "mask_in": mask[c],
            "wa1_in": wa1, "wa2_in": wa2, "w3_in": w3, "c3_in": c3,
            "b1_in": b1r, "b2_in": b2r,
            "ident_in": np.eye(P, dtype=np.float32),
        })

    if Tg not in _CACHE:
        _CACHE[Tg] = _build(Tg)
    nc = _CACHE[Tg]
    res = run_bass_kernel_spmd(nc, in_maps, list(range(NCORES)))

    out = np.empty(N_NODES, np.float32)
    for c in range(NCORES):
        o = np.asarray(res.results[c]["out_p"])      # [P, NBLK]
        r = (np.arange(NBLK)[:, None] * 1024 + c * P + np.arange(P)[None, :])
        nodes = order[r.reshape(-1)]
        vals = o.T.reshape(-1)                       # slot-major: g*P + p
        valid = nodes < N_NODES
        out[nodes[valid]] = vals[valid]
    return out
